# revision 12
# baseline (speedup 1.0000x reference)
"""DeepFM forward on 8 Trainium2 NeuronCores (Bass/Tile, SPMD data-parallel).

Strategy
--------
The dominant cost is streaming categorical_feats [4096, 26000] f32 (426 MB)
from HBM; everything else is tiny. Batch is sharded 8 ways (512 rows/core),
embedding tables and MLP weights are replicated.

Per core:
  * Each categorical feature is a one-hot block of 1000 = 8 * 125. The
    stream is DMA-loaded with an inline f32->f16 cast (0/1 is exact).
  * Weighted fold pyramid: each block is folded 1000 -> 500 -> 250 -> 125
    with out = lo + w*hi (w = 2, 4, 16). The single hot element survives at
    position r = label mod 125 with value 2^m where m bit-reverse-encodes
    q = label div 125. This compresses the stream 8x using three
    elementwise DVE passes over successively halved data.
  * The folded [128, 26*125] tiles are PE-transposed ([125, 512] per block)
    and contracted with a tiny host-built [125, 52] matrix per block
    ([iota | ones] selectors) to produce per-row A = r * 2^m and B = 2^m in
    natural batch layout. Then r = A / B (exact: B is a power of two),
    125*q via a 129-entry gather table T[B], has = min(B, 1). B = 0
    (all-zero block) decodes to label 0 with has = 0, matching the
    reference argmax while zeroing the FM-linear contribution.
  * offsets = r + 125*q + 1000*c drive one indirect-DMA gather per batch
    tile from a host-fused table W_big[26000, 17] = [emb_tables | w_fm_cat]
    (the w_fm column is masked by `has`).
  * The gathered rows are PE-transposed into a [442, 512] embedT layout and
    the whole DeepFM tail (MLP + FM bilinear + FM linear + sigmoid) runs as
    a handful of small transposed matmuls (float32r fast path) with
    host-prefused weights.

All engines stay well under the ~148 us/core HBM streaming floor, so the
kernel is memory-bound as intended.
"""

import numpy as np

import concourse.bass as bass
import concourse.mybir as mybir
import concourse.tile as tile
from concourse import bacc
from concourse.bass_interp import get_hw_module
from concourse.bass_utils import run_bass_kernel_spmd

N_CORES = 8
B = 4096
NUM = 13
NCAT = 26
V = 1000
E = 16
H1, H2 = 512, 256
BS = B // N_CORES          # 512 rows per core
NT = BS // 128             # 4 batch tiles of 128 rows
CHUNK = 13 * V             # 13000 columns per streaming chunk (13 blocks)
NBLK = CHUNK // V          # 13
NCHUNK = (NCAT * V) // CHUNK  # 2 chunks per batch tile
VF = 125                   # folded block width
CATW = NCAT * 17           # 442: gathered row width per batch row
# K-chunks of the 442-row embedT, aligned to whole features (7/7/7/5)
KCH = [(0, 119), (119, 119), (238, 119), (357, 85)]

F32 = mybir.dt.float32
F32R = mybir.dt.float32r
F16 = mybir.dt.float16
I16 = mybir.dt.int16
I32 = mybir.dt.int32
AF = mybir.ActivationFunctionType
OP = mybir.AluOpType

# packed f32 constant layout: name -> columns (all padded to 128 partitions)
_WPACK = [
    ("ident", 128),
    ("cbase", NCAT),
    ("w1c", 4 * H1),
    ("ssel", 4 * 17),
    ("a13", H1),
    ("b1p", H1 // 128),
    ("w2", 4 * H2),
    ("b2p", H2 // 128),
    ("w3", 2),
    ("b3s", 1),
    ("wnum", E),
    ("bnsp", 1),
    ("wfm13", 1),
    ("bfm", 1),
    ("ones16", 1),
]
_WOFF = {}
_off = 0
for _n, _w in _WPACK:
    _WOFF[_n] = _off
    _off += _w
WPACK_F = _off

_HOFF = {"identh": 0, "abw": 128}
HPACK_F = 128 + NCAT * 52


def _build_program():
    nc = bacc.Bacc("TRN2", target_bir_lowering=False, debug=False,
                   enable_asserts=True, num_devices=N_CORES)

    def din(name, shape, dtype=F32):
        return nc.dram_tensor(name, list(shape), dtype, kind="ExternalInput").ap()

    xnum = din("xnum", [BS, NUM])
    cat = din("cat", [BS, NCAT * V])
    wpack = din("wpack", [128, WPACK_F])
    hpack = din("hpack", [128, HPACK_F], F16)
    wbig = din("wbig", [NCAT * V, 64])   # [emb | w_fm | pad] rows (256B for dma_gather)
    yout = nc.dram_tensor("y", [1, BS], F32, kind="ExternalOutput").ap()

    with tile.TileContext(nc) as tc:
        with (
            tc.tile_pool(name="big", bufs=3) as bigpool,
            tc.tile_pool(name="pyr", bufs=1) as pyr,
            tc.tile_pool(name="cst", bufs=1) as cst,
            tc.tile_pool(name="sm", bufs=1) as sm,
            tc.tile_pool(name="pst", bufs=3, space="PSUM") as pst_pool,
            tc.tile_pool(name="pab", bufs=2, space="PSUM") as pab_pool,
            tc.tile_pool(name="pmm", bufs=3, space="PSUM") as pmm_pool,
        ):
            # ---- resident constants / weights (two packed DMAs) ----
            wp = cst.tile([128, WPACK_F], F32, tag="wp")
            nc.sync.dma_start(out=wp[:], in_=wpack[:])
            hp = cst.tile([128, HPACK_F], F16, tag="hp")
            nc.sync.dma_start(out=hp[:], in_=hpack[:])

            def wslice(name, rows=128, w=None, k=0, kw=0):
                o = _WOFF[name] + k * kw
                if w is None:
                    w = dict(_WPACK)[name]
                return wp[0:rows, o:o + w]

            ident_t = wslice("ident")
            cbase_t = wslice("cbase")
            w1c_t = [wslice("w1c", rows=kk, w=H1, k=k, kw=H1)
                     for k, (k0, kk) in enumerate(KCH)]
            ssel_t = [wslice("ssel", rows=kk, w=17, k=k, kw=17)
                      for k, (k0, kk) in enumerate(KCH)]
            a13_t = wslice("a13", rows=NUM)
            b1p_t = wslice("b1p")
            w2_t = [wslice("w2", w=H2, k=k, kw=H2) for k in range(4)]
            b2p_t = wslice("b2p")
            w3_t = [wslice("w3", w=1, k=k, kw=1) for k in range(2)]
            b3s_t = wslice("b3s", rows=1)
            wnum_t = wslice("wnum", rows=NUM)
            bnsp_t = wslice("bnsp", rows=E)
            wfm13_t = wslice("wfm13", rows=NUM)
            bfm_t = wslice("bfm", rows=1)
            ones16_t = wslice("ones16", rows=E)
            identh_t = hp[:, 0:128]
            abw_t = hp[0:VF, 128:128 + NCAT * 52]

            catT = [cst.tile([kk, BS], F32, tag=f"catT{k}", name=f"catT{k}")
                    for k, (k0, kk) in enumerate(KCH)]
            xT = cst.tile([NUM, BS], F32, tag="xT")

            # ---- numeric features: load + transpose to [13, BS] ----
            for t in range(NT):
                xn = sm.tile([128, NUM], F32, tag=f"xn{t}", name=f"xn{t}")
                nc.sync.dma_start(out=xn[:], in_=xnum[t * 128:(t + 1) * 128, :])
                psx = pst_pool.tile([NUM, 128], F32, tag="pst", name=f"psx{t}")
                nc.tensor.transpose(out=psx[:], in_=xn[:], identity=ident_t)
                nc.scalar.copy(out=xT[:, t * 128:(t + 1) * 128], in_=psx[:])

            # ---- heavy stream: weighted fold pyramid ----
            for t in range(NT):
                fbuf = pyr.tile([128, NCAT * VF], F16, tag="fbuf", bufs=2,
                                name=f"fbuf{t}")
                fbv = fbuf[:].rearrange("p (c v) -> p c v", v=VF)
                for h in range(NCHUNK):
                    oh = bigpool.tile([128, CHUNK], F16, tag="oh", name=f"oh{t}{h}")
                    nc.gpsimd.dma_start(
                        out=oh[:],
                        in_=cat[t * 128:(t + 1) * 128, h * CHUNK:(h + 1) * CHUNK])
                    ohv = oh[:].rearrange("p (c v) -> p c v", v=V)
                    l1 = pyr.tile([128, NBLK * 500], F16, tag="l1", name=f"l1_{t}{h}")
                    l1v = l1[:].rearrange("p (c v) -> p c v", v=500)
                    nc.vector.scalar_tensor_tensor(
                        out=l1v, in0=ohv[:, :, 500:1000], scalar=16.0,
                        in1=ohv[:, :, 0:500], op0=OP.mult, op1=OP.add)
                    l2 = pyr.tile([128, NBLK * 250], F16, tag="l2", name=f"l2_{t}{h}")
                    l2v = l2[:].rearrange("p (c v) -> p c v", v=250)
                    nc.vector.scalar_tensor_tensor(
                        out=l2v, in0=l1v[:, :, 250:500], scalar=4.0,
                        in1=l1v[:, :, 0:250], op0=OP.mult, op1=OP.add)
                    nc.vector.scalar_tensor_tensor(
                        out=fbv[:, h * NBLK:(h + 1) * NBLK, :],
                        in0=l2v[:, :, 125:250], scalar=2.0,
                        in1=l2v[:, :, 0:125], op0=OP.mult, op1=OP.add)

                # transpose folded blocks into ftile [125, 26*128]
                ftile = sm.tile([VF, NCAT * 128], F16, tag="ftile", bufs=2,
                                name=f"ftile{t}")
                for c in range(NCAT):
                    pst = pst_pool.tile([VF, 128], F16, tag="pst", name=f"pf{t}{c}")
                    nc.tensor.transpose(out=pst[:], in_=fbuf[:, c * VF:(c + 1) * VF],
                                        identity=identh_t)
                    nc.scalar.copy(out=ftile[:, c * 128:(c + 1) * 128], in_=pst[:])

                # ---- decode labels, gather, build embedT (per batch tile) ----
                abps = pab_pool.tile([128, 52], F32, tag="pab", name=f"abps{t}")
                for c in range(NCAT):
                    nc.tensor.matmul(
                        out=abps[:], lhsT=ftile[:, c * 128:(c + 1) * 128],
                        rhs=abw_t[:, c * 52:(c + 1) * 52],
                        start=(c == 0), stop=(c == NCAT - 1))
                ab = sm.tile([128, 52], F32, tag=f"ab{t}", name=f"ab{t}")
                nc.scalar.copy(out=ab[:], in_=abps[:])
                Av = ab[:, 0:NCAT]
                Bv = ab[:, NCAT:2 * NCAT]

                # r = A / B (B = 2^q, exact)
                bm = sm.tile([128, NCAT], F32, tag=f"bm{t}", name=f"bm{t}")
                nc.vector.tensor_scalar_max(bm[:], Bv, 1.0)
                nc.vector.reciprocal(bm[:], bm[:])
                offs = sm.tile([128, NCAT], F32, tag=f"offs{t}", name=f"offs{t}")
                nc.vector.tensor_tensor(out=offs[:], in0=Av, in1=bm[:], op=OP.mult)

                # 125*q from the f32 exponent bits: (bits(B)>>23 - 127)*125
                bvf = sm.tile([128, NCAT], F32, tag=f"bvf{t}", name=f"bvf{t}")
                nc.vector.tensor_copy(out=bvf[:], in_=Bv.bitcast(I32))
                q125 = sm.tile([128, NCAT], F32, tag=f"q125_{t}", name=f"q125_{t}")
                nc.vector.tensor_scalar(q125[:], bvf[:], float(VF) / (1 << 23),
                                        -127.0 * VF, op0=OP.mult, op1=OP.add)
                nc.vector.tensor_scalar_max(q125[:], q125[:], 0.0)

                nc.vector.tensor_tensor(out=offs[:], in0=offs[:], in1=q125[:],
                                        op=OP.add)
                nc.vector.tensor_tensor(out=offs[:], in0=offs[:], in1=cbase_t,
                                        op=OP.add)
                lab16 = sm.tile([128, NCAT], I16, tag=f"lab16{t}", name=f"lab16{t}")
                nc.vector.tensor_copy(out=lab16[:], in_=offs[:])

                # build wrapped+replicated idx tensor for dma_gather
                idxs = sm.tile([16, NCAT * 8], I16, tag=f"idxs{t}", name=f"idxs{t}")
                idxv = idxs[:].rearrange("q (c r) -> q c r", r=8)
                for r in range(8):
                    nc.sync.dma_start(out=idxv[:, :, r],
                                      in_=lab16[r * 16:(r + 1) * 16, :])
                dscr = nc.dram_tensor(f"dscr{t}", [16, NCAT * 8], I16,
                                      kind="Internal").ap()
                nc.sync.dma_start(out=dscr[:], in_=idxs[:])
                idxs128 = sm.tile([128, NCAT * 8], I16, tag=f"idxs128_{t}",
                                  name=f"idxs128_{t}")
                bcast = bass.AP(dscr.tensor, 0,
                                [[0, 8], [NCAT * 8, 16], [1, NCAT * 8]])
                nc.sync.dma_start(out=idxs128[:], in_=bcast)

                # gather [emb | w_fm | pad] rows: [128, 26, 64]
                dst = sm.tile([128, NCAT * 64], F32, tag="dst", bufs=2,
                              name=f"dst{t}")
                dstv = dst[:].rearrange("p (c e) -> p c e", e=64)
                nc.gpsimd.dma_gather(
                    out_ap=dstv, in_ap=wbig[:], idxs_ap=idxs128[:],
                    num_idxs=NCAT * 128, num_idxs_reg=NCAT * 128,
                    elem_size=64, single_packet=False)

                # pack gathered rows to [128, 442] and mask w_fm by presence
                cat17 = sm.tile([128, CATW], F32, tag="cat17", bufs=2,
                                name=f"cat17_{t}")
                c17v = cat17[:].rearrange("p (c e) -> p c e", e=17)
                nc.vector.tensor_copy(out=c17v, in_=dstv[:, :, 0:17])
                has01 = sm.tile([128, NCAT], F32, tag=f"has{t}", name=f"has{t}")
                nc.vector.tensor_scalar_min(has01[:], Bv, 1.0)
                nc.vector.tensor_tensor(out=c17v[:, :, 16], in0=c17v[:, :, 16],
                                        in1=has01[:], op=OP.mult)

                # transpose to embedT layout [442, BS] (feature-aligned chunks)
                for k, (k0, kk) in enumerate(KCH):
                    pst = pst_pool.tile([kk, 128], F32, tag="pst", name=f"pc{t}{k}")
                    nc.tensor.transpose(out=pst[:],
                                        in_=cat17[:, k0:k0 + kk],
                                        identity=ident_t)
                    nc.scalar.copy(out=catT[k][:, t * 128:(t + 1) * 128],
                                   in_=pst[:])

            def rmm(out, lhsT, rhs, **kw):
                nc.tensor.matmul(out=out, lhsT=lhsT, rhs=rhs, **kw)

            # ---- FM branch ----
            fmps = pmm_pool.tile([E, BS], F32, tag="pmm", name="fmps")
            for k in range(4):
                rmm(fmps[:], ssel_t[k][:, 0:E], catT[k][:],
                    start=(k == 0), stop=(k == 3))
            nsps = pmm_pool.tile([E, BS], F32, tag="pmm", name="nsps")
            rmm(nsps[:], wnum_t, xT[:], start=True, stop=True)
            numsum = sm.tile([E, BS], F32, tag="numsum")
            nc.scalar.activation(out=numsum[:], in_=nsps[:], func=AF.Identity,
                                 bias=bnsp_t[:, 0:1], scale=1.0)
            prodfm = sm.tile([E, BS], F32, tag="prodfm")
            nc.vector.tensor_mul(out=prodfm[:], in0=numsum[:], in1=fmps[:])
            # yfm = sum(prodfm) + x @ wfm13 + sum_c wfm_gathered  (one psum group)
            yfmps = pmm_pool.tile([1, BS], F32, tag="pmm", name="yfmps")
            for k in range(4):
                rmm(yfmps[:], ssel_t[k][:, E:E + 1], catT[k][:],
                    start=(k == 0), stop=False)
            rmm(yfmps[:], wfm13_t, xT[:], start=False, stop=False)
            rmm(yfmps[:], ones16_t, prodfm[:], start=False, stop=True)
            yfm = yfmps

            # ---- deep branch ----
            h1s = []
            for g in range(H1 // 128):
                h1ps = pmm_pool.tile([128, BS], F32, tag="pmm", name=f"h1ps{g}")
                for k, (k0, kk) in enumerate(KCH):
                    rmm(h1ps[:], w1c_t[k][:, g * 128:(g + 1) * 128],
                        catT[k][:], start=(k == 0), stop=False)
                rmm(h1ps[:], a13_t[:, g * 128:(g + 1) * 128],
                    xT[:], start=False, stop=True)
                hs = sm.tile([128, BS], F32, tag=f"h1s{g}", name=f"h1s{g}")
                nc.scalar.activation(out=hs[:], in_=h1ps[:], func=AF.Relu,
                                     bias=b1p_t[:, g:g + 1], scale=1.0)
                h1s.append(hs)
            h2s = []
            for g in range(H2 // 128):
                h2ps = pmm_pool.tile([128, BS], F32, tag="pmm", name=f"h2ps{g}")
                for k in range(4):
                    rmm(h2ps[:], w2_t[k][:, g * 128:(g + 1) * 128],
                        h1s[k][:], start=(k == 0), stop=(k == 3))
                hs = sm.tile([128, BS], F32, tag=f"h2s{g}", name=f"h2s{g}")
                nc.scalar.activation(out=hs[:], in_=h2ps[:], func=AF.Relu,
                                     bias=b2p_t[:, g:g + 1], scale=1.0)
                h2s.append(hs)
            ydps = pmm_pool.tile([1, BS], F32, tag="pmm", name="ydps")
            for k in range(2):
                rmm(ydps[:], w3_t[k], h2s[k][:], start=(k == 0), stop=(k == 1))
            ydeep = sm.tile([1, BS], F32, tag="ydeep")
            nc.scalar.activation(out=ydeep[:], in_=ydps[:], func=AF.Relu,
                                 bias=b3s_t[:, 0:1], scale=1.0)

            # ---- combine + sigmoid ----
            tsum = sm.tile([1, BS], F32, tag="tsum")
            nc.vector.tensor_add(out=tsum[:], in0=yfm[:], in1=ydeep[:])
            yo = sm.tile([1, BS], F32, tag="yo")
            nc.scalar.activation(out=yo[:], in_=tsum[:], func=AF.Sigmoid,
                                 bias=bfm_t[:, 0:1], scale=1.0)
            nc.sync.dma_start(out=yout[:], in_=yo[:])

    nc.compile()
    return nc


def _host_weights(W_num, b_num, emb_tables, w_fm, b_fm, W1, b1, W2, b2, W3, b3):
    f32 = np.float32
    emb = np.ascontiguousarray(np.asarray(emb_tables, f32))       # [26,1000,16]
    wfm = np.asarray(w_fm, f32).reshape(-1)                        # [13+26000]
    W1 = np.asarray(W1, f32)
    W_num = np.asarray(W_num, f32)
    b_num = np.asarray(b_num, f32)

    wbig = np.zeros((NCAT * V, 64), f32)
    wbig[:, 0:E] = emb.reshape(NCAT * V, E)
    wbig[:, E] = wfm[NUM:]

    w1c = np.zeros((CATW, H1), f32)
    W1cat = W1[NUM * E:]                                           # [416, 512]
    for c in range(NCAT):
        w1c[17 * c:17 * c + 16] = W1cat[16 * c:16 * c + 16]

    a13 = np.einsum("ie,ien->in", W_num, W1[:NUM * E].reshape(NUM, E, H1))
    b1f = np.asarray(b1, f32) + b_num.reshape(NUM * E) @ W1[:NUM * E]

    ssel = np.zeros((CATW, 17), f32)
    ssel[np.arange(CATW), np.arange(CATW) % 17] = 1.0

    # ---- pack f32 constants into wpack [128, WPACK_F] ----
    wpack = np.zeros((128, WPACK_F), f32)

    def put(name, arr, k=0, kw=0):
        arr = np.asarray(arr, f32)
        o = _WOFF[name] + k * kw
        wpack[:arr.shape[0], o:o + arr.shape[1]] = arr

    put("ident", np.eye(128, dtype=f32))
    put("cbase", np.tile(np.arange(NCAT, dtype=f32) * V, (128, 1)))
    for k, (k0, kk) in enumerate(KCH):
        put("w1c", w1c[k0:k0 + kk], k=k, kw=H1)
        put("ssel", ssel[k0:k0 + kk], k=k, kw=17)
    put("a13", a13)
    put("b1p", b1f.reshape(H1 // 128, 128).T)
    W2a = np.asarray(W2, f32)
    for k in range(4):
        put("w2", W2a[k * 128:(k + 1) * 128], k=k, kw=H2)
    put("b2p", np.asarray(b2, f32).reshape(H2 // 128, 128).T)
    W3a = np.asarray(W3, f32).reshape(H2, 1)
    for k in range(2):
        put("w3", W3a[k * 128:(k + 1) * 128], k=k, kw=1)
    put("b3s", np.asarray(b3, f32).reshape(1, 1))
    put("wnum", W_num)
    put("bnsp", b_num.sum(axis=0).reshape(E, 1))
    put("wfm13", wfm[:NUM].reshape(NUM, 1))
    put("bfm", np.asarray(b_fm, f32).reshape(1, 1))
    put("ones16", np.ones((E, 1), f32))

    # ---- pack fp16 constants into hpack [128, HPACK_F] ----
    hpack = np.zeros((128, HPACK_F), np.float16)
    hpack[:, 0:128] = np.eye(128, dtype=np.float16)
    abw = np.zeros((VF, NCAT * 52), np.float16)
    for c in range(NCAT):
        abw[:, c * 52 + c] = np.arange(VF, dtype=np.float16)
        abw[:, c * 52 + NCAT + c] = 1.0
    hpack[:VF, 128:128 + NCAT * 52] = abw

    return {
        "wpack": wpack,
        "hpack": hpack,
        "wbig": np.ascontiguousarray(wbig),
    }


def make_in_maps(**inputs):
    """Build the per-core input maps (also used by test.py for CoreSim)."""
    xnum = np.ascontiguousarray(np.asarray(inputs["numeric_feats"], np.float32))
    cat = np.asarray(inputs["categorical_feats"], np.float32)
    shared = _host_weights(
        inputs["W_num"], inputs["b_num"], inputs["emb_tables"],
        inputs["w_fm"], inputs["b_fm"], inputs["W1"], inputs["b1"],
        inputs["W2"], inputs["b2"], inputs["W3"], inputs["b3"])
    in_maps = []
    for i in range(N_CORES):
        m = dict(shared)
        m["xnum"] = xnum[i * BS:(i + 1) * BS]
        m["cat"] = np.ascontiguousarray(cat[i * BS:(i + 1) * BS])
        in_maps.append(m)
    return in_maps


def kernel(**inputs):
    in_maps = make_in_maps(**inputs)
    nc = _build_program()
    nc.m = get_hw_module(nc.m)
    res = run_bass_kernel_spmd(nc, in_maps, core_ids=list(range(N_CORES)))
    out = np.concatenate(
        [res.results[i]["y"].reshape(BS, 1) for i in range(N_CORES)], axis=0)
    return out.astype(np.float32)


if __name__ == "__main__":
    prog = _build_program()
    print("program built ok:",
          sum(len(b.instructions) for f in prog.m.functions for b in f.blocks),
          "instructions")
